# revision 5
# baseline (speedup 1.0000x reference)
"""Trainium2 Bass kernel for a 2-layer GCN (PyG GCNConv semantics), v3.

Full-input contract: kernel(**inputs) takes the complete (unsharded) numpy
inputs and returns the full [N, OUT] float32 output.  Nodes are sharded
across 8 NeuronCores (1D graph partition, LPT-balanced by in-degree);
transformed node features are AllGathered between layers.

Key structure (vs a naive gather kernel):
  - Selection matrices S (one dst-slot one-hot column per edge, entries =
    edge multiplicity) are HOST-precomputed in fp8e4 (0/1/2.. exact) and
    streamed from DRAM; aggregation is one 128-edge matmul per tile.
  - Symmetric normalization is factorized: z~ = dinv * (x @ W1) via a
    per-partition activation scale in phase A, and the dst-side dinv is
    applied in the post-aggregation combines/relu (relu(dinv*x) =
    dinv*relu(x) since dinv > 0).  S stays pure-integer so fp8 is exact.
  - Self-loops never enter the gather stream; the (A+I) diagonal term is a
    per-block matmul against a resident fp8 identity tile.
  - dma_gather descriptor generation is spread over SWDGE queues 1-3
    (queue 0 blocks GPSIMD dispatch), ~3x overlap on the Q7 core pairs.
  - Gather calls pack whole (half, block) groups and trim trailing pad rows
    via the runtime index count; source indices are split at row 32768 into
    two table views to satisfy dma_gather's int16 indexing.
  - Per-block finalize callbacks run inside the aggregation tile stream, so
    relu/h-DMA (L1) and the W2 transform + output DMA (L2) interleave with
    gathers instead of trailing them.

Math:  h~  = dinv * relu(dinv * (A+I) z~ + b1)      z~ = dinv * (x @ W1)
       out = dinv * ((A+I) h~) @ W2 (+ b2)
with deg counted at target incl. self-loop, dinv = deg^-1/2.
"""

import numpy as np
import ml_dtypes

BF16 = ml_dtypes.bfloat16
FP8 = ml_dtypes.float8_e4m3fn

# ---- hardcoded problem constants ----
N_FULL = 50000
F_IN = 256
HID = 128
OUT = 64
NCORES = 8
P = 128
CHUNK = 15         # max 128-edge tiles per gather call
GQUEUES = (1, 2, 3)  # SWDGE queue rotation (q0 excluded: it blocks dispatch)
NAG = 2            # index-range groups (int16 halfsplit)


# ----------------------------------------------------------------------------
# host-side graph preprocessing
# ----------------------------------------------------------------------------

def _balance_nodes(n, nshard, nblk, weight):
    """LPT-balance nodes into ncores*nblk 128-slot blocks by weight."""
    import heapq

    lastcap = nshard - (nblk - 1) * P
    nb = NCORES * nblk
    caps = np.full(nb, P, np.int64)
    caps[nblk - 1 :: nblk] = lastcap
    order = np.argsort(-weight, kind="stable")
    heap = [(0.0, int(b)) for b in range(nb)]
    heapq.heapify(heap)
    members = [[] for _ in range(nb)]
    for node in order:
        while True:
            w, b = heapq.heappop(heap)
            if len(members[b]) < caps[b]:
                members[b].append(node)
                if len(members[b]) < caps[b]:
                    heapq.heappush(heap, (w + weight[node], b))
                break
    perm = np.concatenate([np.asarray(m, np.int64) for m in members])
    pos = np.empty(n, np.int64)
    pos[perm] = np.arange(n)
    return perm, pos


def host_prep(edge_index, n, ncores, chunk):
    """Sort/dedup/pad/shard edges; build fp8 S tiles and int16 gather indices."""
    nshard = n // ncores
    nblk = -(-nshard // P)

    src = np.asarray(edge_index[0], np.int64)
    dst = np.asarray(edge_index[1], np.int64)
    deg = (np.bincount(dst, minlength=n) + 1).astype(np.float64)  # incl self-loop
    dinv = 1.0 / np.sqrt(deg)

    perm, pos = _balance_nodes(n, nshard, nblk, deg)

    halfsplit = 32768  # int16 index limit for dma_gather
    psrc = pos[src]
    psrc_c = (psrc >= halfsplit).astype(np.int64)   # index-range half
    psrc_rel = psrc - psrc_c * halfsplit

    pdst = pos[dst]
    core = pdst // nshard
    dstl = pdst - core * nshard
    blk = dstl // P
    slot = dstl - blk * P

    # dedup multi-edges: unique (core, chunk, blk, slot, src) with counts
    key = ((((core * NAG + psrc_c) * nblk + blk) * P + slot) * n + psrc_rel)
    ukey, mult = np.unique(key, return_counts=True)
    upsrc = ukey % n
    rest = ukey // n
    uslot = rest % P
    rest //= P
    ublk = rest % nblk
    rest //= nblk
    uchunk = rest % NAG
    ucore = rest // NAG

    gkey = (ucore * NAG + uchunk) * nblk + ublk
    counts = np.bincount(gkey, minlength=ncores * NAG * nblk).reshape(
        ncores, NAG, nblk)
    tiles_hb = np.ceil(counts.max(axis=0) / P).astype(np.int64)  # [NAG, nblk]

    order = np.argsort(gkey, kind="stable")
    s_src = upsrc[order]
    s_slot = uslot[order]
    s_mult = mult[order]
    goff = np.zeros(ncores * NAG * nblk + 1, np.int64)
    np.cumsum(counts.reshape(-1), out=goff[1:])

    tile_start = np.zeros((NAG, nblk), np.int64)
    t = 0
    for h in range(NAG):
        for b in range(nblk):
            tile_start[h, b] = t
            t += tiles_hb[h, b]
    t_total = int(t)

    idx_flat = np.zeros((ncores, t_total * P), np.int16)
    stile = np.zeros((ncores, t_total * P, P), np.uint8)  # fp8 bits via view
    sview = stile.view(FP8)
    assert s_mult.max() <= 16, "edge multiplicity exceeds fp8 exact-int range"
    for c in range(ncores):
        for h in range(NAG):
            gbase = (c * NAG + h) * nblk
            for b in range(nblk):
                g0 = goff[gbase + b]
                cnt = counts[c, h, b]
                o0 = tile_start[h, b] * P
                idx_flat[c, o0 : o0 + cnt] = s_src[g0 : g0 + cnt].astype(np.int16)
                sview[c, np.arange(o0, o0 + cnt), s_slot[g0 : g0 + cnt]] = (
                    s_mult[g0 : g0 + cnt].astype(np.float32)
                )

    # wrapped int16 index layout (partition k, col i = idx[16 i + k], x8 groups)
    idx_w = np.empty((ncores, 128, t_total * 8), np.int16)
    for c in range(ncores):
        w = idx_flat[c].reshape(-1, 16).T
        idx_w[c] = np.tile(w, (8, 1))

    # S tiles in [partition=e(128), t_total, slot(128)] layout
    stile_dev = np.ascontiguousarray(
        stile.reshape(ncores, t_total, P, P).transpose(0, 2, 1, 3))

    # per-core dinv packed [slot(128), nblk]
    dinv_pack = np.zeros((ncores, P, nblk), np.float32)
    dperm = dinv[perm]
    for c in range(ncores):
        dloc = np.zeros(nblk * P, np.float64)
        dloc[:nshard] = dperm[c * nshard : (c + 1) * nshard]
        dinv_pack[c] = dloc.reshape(nblk, P).T.astype(np.float32)

    # gather calls: consecutive (chunk, block) groups packed into <=CHUNK-tile
    # calls; the final group's tail is trimmed via the runtime idx count
    maxcnt = counts.max(axis=0)  # [NAG, nblk]
    chunks = []
    for h in range(NAG):
        cur = None  # [start_tile, ntiles, realcnt]
        for b in range(nblk):
            nt = int(tiles_hb[h, b])
            if nt == 0:
                continue
            if cur is not None and cur[1] + nt <= chunk:
                cur[2] = cur[1] * P + int(maxcnt[h, b])
                cur[1] += nt
            else:
                if cur is not None:
                    chunks.append((h, cur[0], cur[1], cur[2]))
                cur = [int(tile_start[h, b]), nt, int(maxcnt[h, b])]
        if cur is not None:
            chunks.append((h, cur[0], cur[1], cur[2]))

    tile_info = []
    for h in range(NAG):
        for b in range(nblk):
            nt = int(tiles_hb[h, b])
            for i in range(nt):
                tile_info.append((h, b, i == 0, i == nt - 1))
    assert len(tile_info) == t_total

    # resident-S prefix: tiles [0, t_res) live in SBUF after layer 1 loads
    # them, so layer 2 skips their re-streaming entirely
    t_res_target = 700
    t_res = 0
    for (h, cstart, cnt, realcnt) in chunks:
        if cstart + cnt <= t_res_target:
            t_res = max(t_res, cstart + cnt)
    struct = dict(
        t_res=t_res,
        n=n, ncores=ncores, nshard=nshard, nblk=nblk,
        lastv=nshard - (nblk - 1) * P, npad=nblk * P,
        t_total=t_total, tiles_hb=tiles_hb, tile_start=tile_start,
        chunks=chunks, tile_info=tile_info, halfsplit=halfsplit,
    )
    percore = dict(idx_w=idx_w, stile=stile_dev, dinv=dinv_pack)
    return struct, percore, perm, pos


# ----------------------------------------------------------------------------
# device program
# ----------------------------------------------------------------------------

def build_program(st, f_in, hid, out_f, has_b1, has_b2):
    import concourse.mybir as mybir
    import concourse.tile as tile
    from concourse import bacc

    dt = mybir.dt
    Alu = mybir.AluOpType
    Act = mybir.ActivationFunctionType

    ncores = st["ncores"]
    nshard, nblk, npad, lastv = st["nshard"], st["nblk"], st["npad"], st["lastv"]
    t_total = st["t_total"]
    chunks = st["chunks"]
    tile_info = st["tile_info"]
    tiles_hb = st["tiles_hb"]
    halfsplit = st["halfsplit"]
    t_res = st["t_res"]
    kt = f_in // P

    nc = bacc.Bacc(
        "TRN2", target_bir_lowering=False, debug=False,
        enable_asserts=False, num_devices=ncores, num_swdge_queues=4,
    )

    xT_d = nc.dram_tensor("xT", [P, kt, npad], dt.bfloat16, kind="ExternalInput")
    w1_d = nc.dram_tensor("w1", [P, kt, hid], dt.bfloat16, kind="ExternalInput")
    w2_d = nc.dram_tensor("w2", [hid, out_f], dt.bfloat16, kind="ExternalInput")
    idx_d = nc.dram_tensor("idx", [128, t_total * 8], dt.int16, kind="ExternalInput")
    stile_d = nc.dram_tensor("stile", [P, t_total, P], dt.float8e4, kind="ExternalInput")
    dinv_d = nc.dram_tensor("dinv", [P, nblk], dt.float32, kind="ExternalInput")
    ident_d = nc.dram_tensor("ident", [P, P], dt.float8e4, kind="ExternalInput")
    if has_b1:
        b1_d = nc.dram_tensor("b1bc", [P, hid], dt.float32, kind="ExternalInput")
    if has_b2:
        b2_d = nc.dram_tensor("b2bc", [P, out_f], dt.float32, kind="ExternalInput")
    out_d = nc.dram_tensor("out", [nshard, out_f], dt.float32, kind="ExternalOutput")

    rg = [list(range(ncores))]

    with tile.TileContext(nc) as tc:
        with (
            tc.tile_pool(name="const", bufs=1) as constp,
            tc.tile_pool(name="stage", bufs=1) as stagep,
            tc.tile_pool(name="dram", bufs=1, space="DRAM") as dramp,
            tc.tile_pool(name="gpool", bufs=7) as gpool,
            tc.tile_pool(name="spool", bufs=2) as spool,
        ):
            w1_sb = constp.tile([P, kt, hid], dt.bfloat16)
            w2_sb = constp.tile([hid, out_f], dt.bfloat16)
            idx_sb = constp.tile([128, t_total * 8], dt.int16)
            dinv_sb = constp.tile([P, nblk], dt.float32)
            ident_sb = constp.tile([P, P], dt.float8e4)
            nc.sync.dma_start(out=w1_sb[:], in_=w1_d[:])
            nc.sync.dma_start(out=w2_sb[:], in_=w2_d[:])
            nc.sync.dma_start(out=idx_sb[:], in_=idx_d[:])
            nc.sync.dma_start(out=dinv_sb[:], in_=dinv_d[:])
            nc.sync.dma_start(out=ident_sb[:], in_=ident_d[:])
            if has_b1:
                b1_sb = constp.tile([P, hid], dt.float32)
                nc.sync.dma_start(out=b1_sb[:], in_=b1_d[:])
            if has_b2:
                b2_sb = constp.tile([P, out_f], dt.float32)
                nc.sync.dma_start(out=b2_sb[:], in_=b2_d[:])

            zstage = stagep.tile([P, npad], dt.bfloat16)   # z~ node-major
            acc = stagep.tile([P, npad], dt.float32)
            hstage = stagep.tile([P, npad], dt.bfloat16)   # h~ node-major


            z_loc = dramp.tile([nshard, hid], dt.bfloat16, name="z_loc")
            h_loc = dramp.tile([nshard, hid], dt.bfloat16, name="h_loc")
            z_full = dramp.tile([st["n"], hid], dt.bfloat16,
                                addr_space="Shared", name="z_full")
            h_full = dramp.tile([st["n"], hid], dt.bfloat16,
                                addr_space="Shared", name="h_full")

            def bts(i, sz):
                return slice(i * sz, (i + 1) * sz)

            def valid(b):
                return lastv if b == nblk - 1 else P

            def dap(b):  # per-partition dinv scalar AP for block b
                return dinv_sb[:, b : b + 1]

            def ag_fire(b, loc, full):
                if b == nblk - 1:
                    nc.gpsimd.collective_compute(
                        "AllGather", mybir.AluOpType.bypass,
                        replica_groups=rg, ins=[loc[:]], outs=[full[:]],
                    )

            gcall = [0]  # rotating gather-queue counter

            # ================= phase A: z~ = dinv * (x @ W1) ================
            with (tc.tile_pool(name="xpool", bufs=1) as xpool,
                  tc.tile_pool(name="pA", bufs=4, space="PSUM") as pA):
                xT_sb = xpool.tile([P, kt, npad], dt.bfloat16)
                nc.sync.dma_start(out=xT_sb[:], in_=xT_d[:])
                for t in range(nblk):
                    ps = pA.tile([P, hid], dt.float32, tag="psA")
                    for k in range(kt):
                        nc.tensor.matmul(
                            out=ps[:], lhsT=xT_sb[:, k, bts(t, P)], rhs=w1_sb[:, k, :],
                            start=(k == 0), stop=(k == kt - 1),
                        )
                    if t % 2 == 0:
                        nc.scalar.activation(out=zstage[:, bts(t, hid)], in_=ps[:],
                                             func=Act.Copy, scale=dap(t))
                    else:
                        nc.vector.tensor_scalar(
                            out=zstage[:, bts(t, hid)], in0=ps[:],
                            scalar1=dap(t), scalar2=None, op0=Alu.mult)
                    v = valid(t)
                    nc.sync.dma_start(out=z_loc[t * P : t * P + v, :],
                                      in_=zstage[:v, bts(t, hid)])
                    ag_fire(t, z_loc, z_full)

            srespool_cm = tc.tile_pool(name="srespool", bufs=1)
            srespool = srespool_cm.__enter__()
            sres = srespool.tile([P, max(t_res, 1), P], dt.float8e4)

            # ================= edge aggregation (shared helper) =============
            def aggregate(layer, tables, stage_for_diag, finalize, pbufs):
                """layer 1: psum[slot,feat] (lhsT=S, rhs=g);
                layer 2: psum[feat,slot] (lhsT=g, rhs=S).  The (A+I) diagonal
                is one identity-matmul injected at each block's first group.
                finalize(b) is emitted right after block b's last combine."""
                with tc.tile_pool(name=f"pB{layer}", bufs=pbufs, space="PSUM") as pB:
                    psd = {}
                    for (h, cstart, cnt, realcnt) in chunks:
                        g = gpool.tile([P, CHUNK, hid], dt.bfloat16, tag="g",
                                       name=f"g{layer}")
                        resident = cstart + cnt <= t_res
                        if resident:
                            sw = sres[:, cstart : cstart + cnt, :]
                            if layer == 1:
                                seng = (nc.sync, nc.scalar)[gcall[0] % 2]
                                seng.dma_start(
                                    out=sw,
                                    in_=stile_d[:, cstart : cstart + cnt, :])
                            swof = cstart
                        else:
                            swt = spool.tile([P, CHUNK, P], dt.float8e4, tag="sw",
                                             name=f"sw{layer}")
                            seng = (nc.sync, nc.scalar)[gcall[0] % 2]
                            seng.dma_start(out=swt[:, :cnt, :],
                                           in_=stile_d[:, cstart : cstart + cnt, :])
                            sw = swt[:, :cnt, :]
                            swof = cstart
                        src_ap = tables[:] if h == 0 else tables[halfsplit:, :]
                        nc.gpsimd.dma_gather(
                            g[:, :cnt, :], src_ap,
                            idx_sb[:, cstart * 8 : (cstart + cnt) * 8],
                            cnt * P, realcnt, hid, single_packet=False,
                            queue_num=GQUEUES[gcall[0] % len(GQUEUES)],
                        )
                        gcall[0] += 1
                        for p in range(cnt):
                            t = cstart + p
                            th, b, first, last = tile_info[t]
                            first_of_block = first and (
                                th == 0 or tiles_hb[0, b] == 0)
                            if first:
                                psd[b] = pB.tile([P, P], dt.float32, tag="psB",
                                                 name=f"ps{layer}")
                            if first_of_block:
                                # (A+I) diagonal: identity x local stage slice
                                if layer == 1:
                                    nc.tensor.matmul(
                                        out=psd[b][:, :hid], lhsT=ident_sb[:],
                                        rhs=stage_for_diag[:, bts(b, hid)],
                                        start=True, stop=False)
                                else:
                                    nc.tensor.matmul(
                                        out=psd[b][:],
                                        lhsT=stage_for_diag[:, bts(b, hid)],
                                        rhs=ident_sb[:],
                                        start=True, stop=False)
                            if layer == 1:
                                nc.tensor.matmul(
                                    out=psd[b][:, :hid], lhsT=sw[:, p, :],
                                    rhs=g[:, p, :],
                                    start=not first_of_block and first, stop=last)
                            else:
                                nc.tensor.matmul(
                                    out=psd[b][:], lhsT=g[:, p, :],
                                    rhs=sw[:, p, :],
                                    start=not first_of_block and first, stop=last)
                            if last:
                                first_group = th == 0 or tiles_hb[0, b] == 0
                                last_group = (
                                    th == NAG - 1
                                    or tiles_hb[th + 1 :, b].sum() == 0)
                                if layer == 1:
                                    # dst-side dinv folded into the combine
                                    if first_group:
                                        nc.scalar.activation(
                                            out=acc[:, bts(b, P)], in_=psd[b][:],
                                            func=Act.Copy, scale=dap(b))
                                    else:
                                        nc.vector.scalar_tensor_tensor(
                                            out=acc[:, bts(b, P)], in0=psd[b][:],
                                            scalar=dap(b), in1=acc[:, bts(b, P)],
                                            op0=Alu.mult, op1=Alu.add)
                                else:
                                    if first_group:
                                        nc.scalar.copy(out=acc[:, bts(b, P)],
                                                       in_=psd[b][:])
                                    else:
                                        nc.vector.tensor_tensor(
                                            out=acc[:, bts(b, P)], in0=psd[b][:],
                                            in1=acc[:, bts(b, P)], op=Alu.add)
                                del psd[b]
                                if last_group:
                                    finalize(b)

            # ================= phase B: L1 aggregation + relu ===============
            def finalize1(b):
                if has_b1:
                    nc.vector.tensor_tensor(out=acc[:, bts(b, P)],
                                            in0=acc[:, bts(b, P)],
                                            in1=b1_sb[:], op=Alu.add)
                nc.scalar.activation(out=hstage[:, bts(b, P)], in_=acc[:, bts(b, P)],
                                     func=Act.Relu, scale=dap(b))
                v = valid(b)
                nc.sync.dma_start(out=h_loc[b * P : b * P + v, :],
                                  in_=hstage[:v, bts(b, P)])
                ag_fire(b, h_loc, h_full)

            aggregate(1, z_full, zstage, finalize1, pbufs=8)

            # ========= phase C+D: L2 aggregation, out = dinv*(agg @ W2) =====
            with (tc.tile_pool(name="pD", bufs=2, space="PSUM") as pD,
                  tc.tile_pool(name="aggp", bufs=4) as aggp,
                  tc.tile_pool(name="outp", bufs=4) as outp):
                def finalize2(t):
                    aggT = aggp.tile([P, P], dt.bfloat16, tag="aggT")
                    outst = outp.tile([P, out_f], dt.float32, tag="outst")
                    nc.scalar.copy(out=aggT[:], in_=acc[:, bts(t, P)])
                    ps = pD.tile([P, out_f], dt.float32, tag="psD")
                    nc.tensor.matmul(out=ps[:], lhsT=aggT[:], rhs=w2_sb[:],
                                     start=True, stop=True)
                    nc.scalar.activation(out=outst[:], in_=ps[:],
                                         func=Act.Copy, scale=dap(t))
                    if has_b2:
                        nc.vector.tensor_tensor(
                            out=outst[:], in0=outst[:],
                            in1=b2_sb[:], op=Alu.add)
                    v = valid(t)
                    nc.sync.dma_start(out=out_d[t * P : t * P + v, :],
                                      in_=outst[:v, :])

                aggregate(2, h_full, hstage, finalize2, pbufs=6)

            srespool_cm.__exit__(None, None, None)

    nc.compile()
    return nc


# ----------------------------------------------------------------------------
# input packing
# ----------------------------------------------------------------------------

def pack_inputs(x, W1, b1, W2, b2, st, percore, perm):
    ncores, nshard, npad = st["ncores"], st["nshard"], st["npad"]
    kt = x.shape[1] // P
    hid = W1.shape[1]
    out_f = W2.shape[1]
    has_b1 = bool(np.any(b1))
    has_b2 = bool(np.any(b2))

    w1h = np.ascontiguousarray(W1.reshape(kt, P, hid).transpose(1, 0, 2)).astype(BF16)
    w2h = np.ascontiguousarray(W2).astype(BF16)
    ident = np.eye(P, dtype=np.float32).astype(FP8)

    xp = x[perm]
    in_maps = []
    for c in range(ncores):
        xpad = np.zeros((npad, kt * P), np.float32)
        xpad[:nshard] = xp[c * nshard : (c + 1) * nshard]
        xT = np.ascontiguousarray(
            xpad.T.reshape(kt, P, npad).transpose(1, 0, 2)).astype(BF16)
        m = {
            "xT": xT, "w1": w1h, "w2": w2h, "ident": ident,
            "idx": np.ascontiguousarray(percore["idx_w"][c]),
            "stile": np.ascontiguousarray(percore["stile"][c]).view(FP8),
            "dinv": np.ascontiguousarray(percore["dinv"][c]),
        }
        if has_b1:
            m["b1bc"] = np.ascontiguousarray(
                np.broadcast_to(b1, (P, hid))).astype(np.float32)
        if has_b2:
            m["b2bc"] = np.ascontiguousarray(
                np.broadcast_to(b2, (P, out_f))).astype(np.float32)
        in_maps.append(m)
    return in_maps, has_b1, has_b2


# ----------------------------------------------------------------------------
# entry point
# ----------------------------------------------------------------------------

_CACHE = {}


def _run(x, edge_index, W1, b1, W2, b2, trace=False):
    from concourse.bass_utils import run_bass_kernel_spmd

    n = x.shape[0]
    st, percore, perm, pos = host_prep(edge_index, n, NCORES, CHUNK)
    in_maps, has_b1, has_b2 = pack_inputs(x, W1, b1, W2, b2, st, percore, perm)

    key = (n, x.shape[1], W1.shape[1], W2.shape[1], st["t_total"],
           tuple(st["tiles_hb"].reshape(-1)), has_b1, has_b2)
    nc = _CACHE.get(key)
    if nc is None:
        nc = build_program(st, x.shape[1], W1.shape[1], W2.shape[1], has_b1, has_b2)
        _CACHE[key] = nc

    res = run_bass_kernel_spmd(nc, in_maps, core_ids=list(range(NCORES)), trace=trace)
    outp = np.concatenate([res.results[c]["out"] for c in range(NCORES)], axis=0)
    out = np.empty_like(outp)
    out[perm] = outp
    return out.astype(np.float32), res


def kernel(x, edge_index, W1, b1, W2, b2):
    out, _ = _run(np.asarray(x, np.float32), np.asarray(edge_index),
                  np.asarray(W1, np.float32), np.asarray(b1, np.float32),
                  np.asarray(W2, np.float32), np.asarray(b2, np.float32))
    return out


# revision 6
# speedup vs baseline: 1.0133x; 1.0133x over previous
"""Trainium2 Bass kernel for a 2-layer GCN (PyG GCNConv semantics), v3.

Full-input contract: kernel(**inputs) takes the complete (unsharded) numpy
inputs and returns the full [N, OUT] float32 output.  Nodes are sharded
across 8 NeuronCores (1D graph partition, LPT-balanced by in-degree);
transformed node features are AllGathered between layers.

Key structure (vs a naive gather kernel):
  - Selection matrices S (one dst-slot one-hot column per edge, entries =
    edge multiplicity) are HOST-precomputed in fp8e4 (0/1/2.. exact) and
    streamed from DRAM; aggregation is one 128-edge matmul per tile.
  - Symmetric normalization is factorized: z~ = dinv * (x @ W1) via a
    per-partition activation scale in phase A, and the dst-side dinv is
    applied in the post-aggregation combines/relu (relu(dinv*x) =
    dinv*relu(x) since dinv > 0).  S stays pure-integer so fp8 is exact.
  - Self-loops never enter the gather stream; the (A+I) diagonal term is a
    per-block matmul against a resident fp8 identity tile.
  - dma_gather descriptor generation is spread over SWDGE queues 1-3
    (queue 0 blocks GPSIMD dispatch), ~3x overlap on the Q7 core pairs.
  - Gather calls pack whole (half, block) groups and trim trailing pad rows
    via the runtime index count; source indices are split at row 32768 into
    two table views to satisfy dma_gather's int16 indexing.
  - Per-block finalize callbacks run inside the aggregation tile stream, so
    relu/h-DMA (L1) and the W2 transform + output DMA (L2) interleave with
    gathers instead of trailing them.

Math:  h~  = dinv * relu(dinv * (A+I) z~ + b1)      z~ = dinv * (x @ W1)
       out = dinv * ((A+I) h~) @ W2 (+ b2)
with deg counted at target incl. self-loop, dinv = deg^-1/2.
"""

import numpy as np
import ml_dtypes

BF16 = ml_dtypes.bfloat16
FP8 = ml_dtypes.float8_e4m3fn

# ---- hardcoded problem constants ----
N_FULL = 50000
F_IN = 256
HID = 128
OUT = 64
NCORES = 8
P = 128
CHUNK = 15         # max 128-edge tiles per gather call
GQUEUES = (1, 2, 3, 0)  # SWDGE queue rotation
NAG = 2            # index-range groups (int16 halfsplit)


# ----------------------------------------------------------------------------
# host-side graph preprocessing
# ----------------------------------------------------------------------------

def _balance_nodes(n, nshard, nblk, weight):
    """LPT-balance nodes into ncores*nblk 128-slot blocks by weight."""
    import heapq

    lastcap = nshard - (nblk - 1) * P
    nb = NCORES * nblk
    caps = np.full(nb, P, np.int64)
    caps[nblk - 1 :: nblk] = lastcap
    order = np.argsort(-weight, kind="stable")
    heap = [(0.0, int(b)) for b in range(nb)]
    heapq.heapify(heap)
    members = [[] for _ in range(nb)]
    for node in order:
        while True:
            w, b = heapq.heappop(heap)
            if len(members[b]) < caps[b]:
                members[b].append(node)
                if len(members[b]) < caps[b]:
                    heapq.heappush(heap, (w + weight[node], b))
                break
    perm = np.concatenate([np.asarray(m, np.int64) for m in members])
    pos = np.empty(n, np.int64)
    pos[perm] = np.arange(n)
    return perm, pos


def host_prep(edge_index, n, ncores, chunk):
    """Sort/dedup/pad/shard edges; build fp8 S tiles and int16 gather indices."""
    nshard = n // ncores
    nblk = -(-nshard // P)

    src = np.asarray(edge_index[0], np.int64)
    dst = np.asarray(edge_index[1], np.int64)
    deg = (np.bincount(dst, minlength=n) + 1).astype(np.float64)  # incl self-loop
    dinv = 1.0 / np.sqrt(deg)

    perm, pos = _balance_nodes(n, nshard, nblk, deg)

    halfsplit = 32768  # int16 index limit for dma_gather
    psrc = pos[src]
    psrc_c = (psrc >= halfsplit).astype(np.int64)   # index-range half
    psrc_rel = psrc - psrc_c * halfsplit

    pdst = pos[dst]
    core = pdst // nshard
    dstl = pdst - core * nshard
    blk = dstl // P
    slot = dstl - blk * P

    # dedup multi-edges: unique (core, chunk, blk, slot, src) with counts
    key = ((((core * NAG + psrc_c) * nblk + blk) * P + slot) * n + psrc_rel)
    ukey, mult = np.unique(key, return_counts=True)
    upsrc = ukey % n
    rest = ukey // n
    uslot = rest % P
    rest //= P
    ublk = rest % nblk
    rest //= nblk
    uchunk = rest % NAG
    ucore = rest // NAG

    gkey = (ucore * NAG + uchunk) * nblk + ublk
    counts = np.bincount(gkey, minlength=ncores * NAG * nblk).reshape(
        ncores, NAG, nblk)
    tiles_hb = np.ceil(counts.max(axis=0) / P).astype(np.int64)  # [NAG, nblk]

    order = np.argsort(gkey, kind="stable")
    s_src = upsrc[order]
    s_slot = uslot[order]
    s_mult = mult[order]
    goff = np.zeros(ncores * NAG * nblk + 1, np.int64)
    np.cumsum(counts.reshape(-1), out=goff[1:])

    tile_start = np.zeros((NAG, nblk), np.int64)
    t = 0
    for h in range(NAG):
        for b in range(nblk):
            tile_start[h, b] = t
            t += tiles_hb[h, b]
    t_total = int(t)

    idx_flat = np.zeros((ncores, t_total * P), np.int16)
    stile = np.zeros((ncores, t_total * P, P), np.uint8)  # fp8 bits via view
    sview = stile.view(FP8)
    assert s_mult.max() <= 16, "edge multiplicity exceeds fp8 exact-int range"
    for c in range(ncores):
        for h in range(NAG):
            gbase = (c * NAG + h) * nblk
            for b in range(nblk):
                g0 = goff[gbase + b]
                cnt = counts[c, h, b]
                o0 = tile_start[h, b] * P
                idx_flat[c, o0 : o0 + cnt] = s_src[g0 : g0 + cnt].astype(np.int16)
                sview[c, np.arange(o0, o0 + cnt), s_slot[g0 : g0 + cnt]] = (
                    s_mult[g0 : g0 + cnt].astype(np.float32)
                )

    # wrapped int16 index layout (partition k, col i = idx[16 i + k], x8 groups)
    idx_w = np.empty((ncores, 128, t_total * 8), np.int16)
    for c in range(ncores):
        w = idx_flat[c].reshape(-1, 16).T
        idx_w[c] = np.tile(w, (8, 1))

    # S tiles in [partition=e(128), t_total, slot(128)] layout
    stile_dev = np.ascontiguousarray(
        stile.reshape(ncores, t_total, P, P).transpose(0, 2, 1, 3))

    # per-core dinv packed [slot(128), nblk]
    dinv_pack = np.zeros((ncores, P, nblk), np.float32)
    dperm = dinv[perm]
    for c in range(ncores):
        dloc = np.zeros(nblk * P, np.float64)
        dloc[:nshard] = dperm[c * nshard : (c + 1) * nshard]
        dinv_pack[c] = dloc.reshape(nblk, P).T.astype(np.float32)

    # gather calls: consecutive (chunk, block) groups packed into <=CHUNK-tile
    # calls; the final group's tail is trimmed via the runtime idx count
    maxcnt = counts.max(axis=0)  # [NAG, nblk]
    chunks = []
    for h in range(NAG):
        cur = None  # [start_tile, ntiles, realcnt]
        for b in range(nblk):
            nt = int(tiles_hb[h, b])
            if nt == 0:
                continue
            if cur is not None and cur[1] + nt <= chunk:
                cur[2] = cur[1] * P + int(maxcnt[h, b])
                cur[1] += nt
            else:
                if cur is not None:
                    chunks.append((h, cur[0], cur[1], cur[2]))
                cur = [int(tile_start[h, b]), nt, int(maxcnt[h, b])]
        if cur is not None:
            chunks.append((h, cur[0], cur[1], cur[2]))

    tile_info = []
    for h in range(NAG):
        for b in range(nblk):
            nt = int(tiles_hb[h, b])
            for i in range(nt):
                tile_info.append((h, b, i == 0, i == nt - 1))
    assert len(tile_info) == t_total

    # resident-S prefix: tiles [0, t_res) live in SBUF after layer 1 loads
    # them, so layer 2 skips their re-streaming entirely
    t_res_target = 700
    t_res = 0
    for (h, cstart, cnt, realcnt) in chunks:
        if cstart + cnt <= t_res_target:
            t_res = max(t_res, cstart + cnt)
    struct = dict(
        t_res=t_res,
        n=n, ncores=ncores, nshard=nshard, nblk=nblk,
        lastv=nshard - (nblk - 1) * P, npad=nblk * P,
        t_total=t_total, tiles_hb=tiles_hb, tile_start=tile_start,
        chunks=chunks, tile_info=tile_info, halfsplit=halfsplit,
    )
    percore = dict(idx_w=idx_w, stile=stile_dev, dinv=dinv_pack)
    return struct, percore, perm, pos


# ----------------------------------------------------------------------------
# device program
# ----------------------------------------------------------------------------

def build_program(st, f_in, hid, out_f, has_b1, has_b2):
    import concourse.mybir as mybir
    import concourse.tile as tile
    from concourse import bacc

    dt = mybir.dt
    Alu = mybir.AluOpType
    Act = mybir.ActivationFunctionType

    ncores = st["ncores"]
    nshard, nblk, npad, lastv = st["nshard"], st["nblk"], st["npad"], st["lastv"]
    t_total = st["t_total"]
    chunks = st["chunks"]
    tile_info = st["tile_info"]
    tiles_hb = st["tiles_hb"]
    halfsplit = st["halfsplit"]
    t_res = st["t_res"]
    kt = f_in // P

    nc = bacc.Bacc(
        "TRN2", target_bir_lowering=False, debug=False,
        enable_asserts=False, num_devices=ncores, num_swdge_queues=4,
    )

    xT_d = nc.dram_tensor("xT", [P, kt, npad], dt.bfloat16, kind="ExternalInput")
    w1_d = nc.dram_tensor("w1", [P, kt, hid], dt.bfloat16, kind="ExternalInput")
    w2_d = nc.dram_tensor("w2", [hid, out_f], dt.bfloat16, kind="ExternalInput")
    idx_d = nc.dram_tensor("idx", [128, t_total * 8], dt.int16, kind="ExternalInput")
    stile_d = nc.dram_tensor("stile", [P, t_total, P], dt.float8e4, kind="ExternalInput")
    dinv_d = nc.dram_tensor("dinv", [P, nblk], dt.float32, kind="ExternalInput")
    ident_d = nc.dram_tensor("ident", [P, P], dt.float8e4, kind="ExternalInput")
    if has_b1:
        b1_d = nc.dram_tensor("b1bc", [P, hid], dt.float32, kind="ExternalInput")
    if has_b2:
        b2_d = nc.dram_tensor("b2bc", [P, out_f], dt.float32, kind="ExternalInput")
    out_d = nc.dram_tensor("out", [nshard, out_f], dt.float32, kind="ExternalOutput")

    rg = [list(range(ncores))]

    with tile.TileContext(nc) as tc:
        with (
            tc.tile_pool(name="const", bufs=1) as constp,
            tc.tile_pool(name="stage", bufs=1) as stagep,
            tc.tile_pool(name="dram", bufs=1, space="DRAM") as dramp,
            tc.tile_pool(name="gpool", bufs=8) as gpool,
            tc.tile_pool(name="spool", bufs=2) as spool,
        ):
            w1_sb = constp.tile([P, kt, hid], dt.bfloat16)
            w2_sb = constp.tile([hid, out_f], dt.bfloat16)
            idx_sb = constp.tile([128, t_total * 8], dt.int16)
            dinv_sb = constp.tile([P, nblk], dt.float32)
            ident_sb = constp.tile([P, P], dt.float8e4)
            nc.sync.dma_start(out=w1_sb[:], in_=w1_d[:])
            nc.sync.dma_start(out=w2_sb[:], in_=w2_d[:])
            nc.sync.dma_start(out=idx_sb[:], in_=idx_d[:])
            nc.sync.dma_start(out=dinv_sb[:], in_=dinv_d[:])
            nc.sync.dma_start(out=ident_sb[:], in_=ident_d[:])
            if has_b1:
                b1_sb = constp.tile([P, hid], dt.float32)
                nc.sync.dma_start(out=b1_sb[:], in_=b1_d[:])
            if has_b2:
                b2_sb = constp.tile([P, out_f], dt.float32)
                nc.sync.dma_start(out=b2_sb[:], in_=b2_d[:])

            zstage = stagep.tile([P, npad], dt.bfloat16)   # z~ node-major
            acc = stagep.tile([P, npad], dt.float32)
            hstage = stagep.tile([P, npad], dt.bfloat16)   # h~ node-major


            z_loc = dramp.tile([nshard, hid], dt.bfloat16, name="z_loc")
            h_loc = dramp.tile([nshard, hid], dt.bfloat16, name="h_loc")
            z_full = dramp.tile([st["n"], hid], dt.bfloat16,
                                addr_space="Shared", name="z_full")
            h_full = dramp.tile([st["n"], hid], dt.bfloat16,
                                addr_space="Shared", name="h_full")

            def bts(i, sz):
                return slice(i * sz, (i + 1) * sz)

            def valid(b):
                return lastv if b == nblk - 1 else P

            def dap(b):  # per-partition dinv scalar AP for block b
                return dinv_sb[:, b : b + 1]

            def ag_fire(b, loc, full):
                if b == nblk - 1:
                    nc.gpsimd.collective_compute(
                        "AllGather", mybir.AluOpType.bypass,
                        replica_groups=rg, ins=[loc[:]], outs=[full[:]],
                    )

            gcall = [0]  # rotating gather-queue counter

            # ================= phase A: z~ = dinv * (x @ W1) ================
            with (tc.tile_pool(name="xpool", bufs=1) as xpool,
                  tc.tile_pool(name="pA", bufs=4, space="PSUM") as pA):
                xT_sb = xpool.tile([P, kt, npad], dt.bfloat16)
                nc.sync.dma_start(out=xT_sb[:], in_=xT_d[:])
                for t in range(nblk):
                    ps = pA.tile([P, hid], dt.float32, tag="psA")
                    for k in range(kt):
                        nc.tensor.matmul(
                            out=ps[:], lhsT=xT_sb[:, k, bts(t, P)], rhs=w1_sb[:, k, :],
                            start=(k == 0), stop=(k == kt - 1),
                        )
                    if t % 2 == 0:
                        nc.scalar.activation(out=zstage[:, bts(t, hid)], in_=ps[:],
                                             func=Act.Copy, scale=dap(t))
                    else:
                        nc.vector.tensor_scalar(
                            out=zstage[:, bts(t, hid)], in0=ps[:],
                            scalar1=dap(t), scalar2=None, op0=Alu.mult)
                    v = valid(t)
                    nc.sync.dma_start(out=z_loc[t * P : t * P + v, :],
                                      in_=zstage[:v, bts(t, hid)])
                    ag_fire(t, z_loc, z_full)

            srespool_cm = tc.tile_pool(name="srespool", bufs=1)
            srespool = srespool_cm.__enter__()
            sres = srespool.tile([P, max(t_res, 1), P], dt.float8e4)

            # ================= edge aggregation (shared helper) =============
            def aggregate(layer, tables, stage_for_diag, finalize, pbufs):
                """layer 1: psum[slot,feat] (lhsT=S, rhs=g);
                layer 2: psum[feat,slot] (lhsT=g, rhs=S).  The (A+I) diagonal
                is one identity-matmul injected at each block's first group.
                finalize(b) is emitted right after block b's last combine."""
                with tc.tile_pool(name=f"pB{layer}", bufs=pbufs, space="PSUM") as pB:
                    psd = {}
                    for (h, cstart, cnt, realcnt) in chunks:
                        g = gpool.tile([P, CHUNK, hid], dt.bfloat16, tag="g",
                                       name=f"g{layer}")
                        resident = cstart + cnt <= t_res
                        if resident:
                            sw = sres[:, cstart : cstart + cnt, :]
                            if layer == 1:
                                seng = (nc.sync, nc.scalar)[gcall[0] % 2]
                                seng.dma_start(
                                    out=sw,
                                    in_=stile_d[:, cstart : cstart + cnt, :])
                            swof = cstart
                        else:
                            swt = spool.tile([P, CHUNK, P], dt.float8e4, tag="sw",
                                             name=f"sw{layer}")
                            seng = (nc.sync, nc.scalar)[gcall[0] % 2]
                            seng.dma_start(out=swt[:, :cnt, :],
                                           in_=stile_d[:, cstart : cstart + cnt, :])
                            sw = swt[:, :cnt, :]
                            swof = cstart
                        src_ap = tables[:] if h == 0 else tables[halfsplit:, :]
                        nc.gpsimd.dma_gather(
                            g[:, :cnt, :], src_ap,
                            idx_sb[:, cstart * 8 : (cstart + cnt) * 8],
                            cnt * P, realcnt, hid, single_packet=False,
                            queue_num=GQUEUES[gcall[0] % len(GQUEUES)],
                        )
                        gcall[0] += 1
                        for p in range(cnt):
                            t = cstart + p
                            th, b, first, last = tile_info[t]
                            first_of_block = first and (
                                th == 0 or tiles_hb[0, b] == 0)
                            if first:
                                psd[b] = pB.tile([P, P], dt.float32, tag="psB",
                                                 name=f"ps{layer}")
                            if first_of_block:
                                # (A+I) diagonal: identity x local stage slice
                                if layer == 1:
                                    nc.tensor.matmul(
                                        out=psd[b][:, :hid], lhsT=ident_sb[:],
                                        rhs=stage_for_diag[:, bts(b, hid)],
                                        start=True, stop=False)
                                else:
                                    nc.tensor.matmul(
                                        out=psd[b][:],
                                        lhsT=stage_for_diag[:, bts(b, hid)],
                                        rhs=ident_sb[:],
                                        start=True, stop=False)
                            if layer == 1:
                                nc.tensor.matmul(
                                    out=psd[b][:, :hid], lhsT=sw[:, p, :],
                                    rhs=g[:, p, :],
                                    start=not first_of_block and first, stop=last)
                            else:
                                nc.tensor.matmul(
                                    out=psd[b][:], lhsT=g[:, p, :],
                                    rhs=sw[:, p, :],
                                    start=not first_of_block and first, stop=last)
                            if last:
                                first_group = th == 0 or tiles_hb[0, b] == 0
                                last_group = (
                                    th == NAG - 1
                                    or tiles_hb[th + 1 :, b].sum() == 0)
                                if layer == 1:
                                    # dst-side dinv folded into the combine
                                    if first_group:
                                        nc.scalar.activation(
                                            out=acc[:, bts(b, P)], in_=psd[b][:],
                                            func=Act.Copy, scale=dap(b))
                                    else:
                                        nc.vector.scalar_tensor_tensor(
                                            out=acc[:, bts(b, P)], in0=psd[b][:],
                                            scalar=dap(b), in1=acc[:, bts(b, P)],
                                            op0=Alu.mult, op1=Alu.add)
                                else:
                                    if first_group:
                                        nc.scalar.copy(out=acc[:, bts(b, P)],
                                                       in_=psd[b][:])
                                    else:
                                        nc.vector.tensor_tensor(
                                            out=acc[:, bts(b, P)], in0=psd[b][:],
                                            in1=acc[:, bts(b, P)], op=Alu.add)
                                del psd[b]
                                if last_group:
                                    finalize(b)

            # ================= phase B: L1 aggregation + relu ===============
            def finalize1(b):
                if has_b1:
                    nc.vector.tensor_tensor(out=acc[:, bts(b, P)],
                                            in0=acc[:, bts(b, P)],
                                            in1=b1_sb[:], op=Alu.add)
                nc.scalar.activation(out=hstage[:, bts(b, P)], in_=acc[:, bts(b, P)],
                                     func=Act.Relu, scale=dap(b))
                v = valid(b)
                nc.sync.dma_start(out=h_loc[b * P : b * P + v, :],
                                  in_=hstage[:v, bts(b, P)])
                ag_fire(b, h_loc, h_full)

            aggregate(1, z_full, zstage, finalize1, pbufs=8)

            # ========= phase C+D: L2 aggregation, out = dinv*(agg @ W2) =====
            with (tc.tile_pool(name="pD", bufs=2, space="PSUM") as pD,
                  tc.tile_pool(name="aggp", bufs=4) as aggp,
                  tc.tile_pool(name="outp", bufs=4) as outp):
                def finalize2(t):
                    aggT = aggp.tile([P, P], dt.bfloat16, tag="aggT")
                    outst = outp.tile([P, out_f], dt.float32, tag="outst")
                    nc.scalar.copy(out=aggT[:], in_=acc[:, bts(t, P)])
                    ps = pD.tile([P, out_f], dt.float32, tag="psD")
                    nc.tensor.matmul(out=ps[:], lhsT=aggT[:], rhs=w2_sb[:],
                                     start=True, stop=True)
                    nc.scalar.activation(out=outst[:], in_=ps[:],
                                         func=Act.Copy, scale=dap(t))
                    if has_b2:
                        nc.vector.tensor_tensor(
                            out=outst[:], in0=outst[:],
                            in1=b2_sb[:], op=Alu.add)
                    v = valid(t)
                    nc.sync.dma_start(out=out_d[t * P : t * P + v, :],
                                      in_=outst[:v, :])

                aggregate(2, h_full, hstage, finalize2, pbufs=6)

            srespool_cm.__exit__(None, None, None)

    nc.compile()
    return nc


# ----------------------------------------------------------------------------
# input packing
# ----------------------------------------------------------------------------

def pack_inputs(x, W1, b1, W2, b2, st, percore, perm):
    ncores, nshard, npad = st["ncores"], st["nshard"], st["npad"]
    kt = x.shape[1] // P
    hid = W1.shape[1]
    out_f = W2.shape[1]
    has_b1 = bool(np.any(b1))
    has_b2 = bool(np.any(b2))

    w1h = np.ascontiguousarray(W1.reshape(kt, P, hid).transpose(1, 0, 2)).astype(BF16)
    w2h = np.ascontiguousarray(W2).astype(BF16)
    ident = np.eye(P, dtype=np.float32).astype(FP8)

    xp = x[perm]
    in_maps = []
    for c in range(ncores):
        xpad = np.zeros((npad, kt * P), np.float32)
        xpad[:nshard] = xp[c * nshard : (c + 1) * nshard]
        xT = np.ascontiguousarray(
            xpad.T.reshape(kt, P, npad).transpose(1, 0, 2)).astype(BF16)
        m = {
            "xT": xT, "w1": w1h, "w2": w2h, "ident": ident,
            "idx": np.ascontiguousarray(percore["idx_w"][c]),
            "stile": np.ascontiguousarray(percore["stile"][c]).view(FP8),
            "dinv": np.ascontiguousarray(percore["dinv"][c]),
        }
        if has_b1:
            m["b1bc"] = np.ascontiguousarray(
                np.broadcast_to(b1, (P, hid))).astype(np.float32)
        if has_b2:
            m["b2bc"] = np.ascontiguousarray(
                np.broadcast_to(b2, (P, out_f))).astype(np.float32)
        in_maps.append(m)
    return in_maps, has_b1, has_b2


# ----------------------------------------------------------------------------
# entry point
# ----------------------------------------------------------------------------

_CACHE = {}


def _run(x, edge_index, W1, b1, W2, b2, trace=False):
    from concourse.bass_utils import run_bass_kernel_spmd

    n = x.shape[0]
    st, percore, perm, pos = host_prep(edge_index, n, NCORES, CHUNK)
    in_maps, has_b1, has_b2 = pack_inputs(x, W1, b1, W2, b2, st, percore, perm)

    key = (n, x.shape[1], W1.shape[1], W2.shape[1], st["t_total"],
           tuple(st["tiles_hb"].reshape(-1)), has_b1, has_b2)
    nc = _CACHE.get(key)
    if nc is None:
        nc = build_program(st, x.shape[1], W1.shape[1], W2.shape[1], has_b1, has_b2)
        _CACHE[key] = nc

    res = run_bass_kernel_spmd(nc, in_maps, core_ids=list(range(NCORES)), trace=trace)
    outp = np.concatenate([res.results[c]["out"] for c in range(NCORES)], axis=0)
    out = np.empty_like(outp)
    out[perm] = outp
    return out.astype(np.float32), res


def kernel(x, edge_index, W1, b1, W2, b2):
    out, _ = _run(np.asarray(x, np.float32), np.asarray(edge_index),
                  np.asarray(W1, np.float32), np.asarray(b1, np.float32),
                  np.asarray(W2, np.float32), np.asarray(b2, np.float32))
    return out
